# revision 1
# baseline (speedup 1.0000x reference)
"""Single-head masked attention (B=4, S=2048, D=1024, fp32) on 8 TRN2 NeuronCores.

Sharding: core c handles batch b=c//2, query half h=c%2 (1024 queries), with
K/V work over all 2048 keys of its batch. For h=1 cores the key axis is
rotated by 1024 on the host so every core runs the identical SPMD program
(attention is invariant to key permutation when the mask is permuted too).

The kernel exploits two algebraic reassociations that cut the matmul work
from 1280 to 1024 tile-matmuls per core:

1) scores^T = K @ Q^T = (x @ Wk^T + bk) @ Q^T
            = x @ (Wk^T @ Q^T)  [+ bk . Q^T, constant per query]
   The bias term is constant across keys for each query, so softmax's shift
   invariance cancels it EXACTLY -- bk is simply dropped. Computing
   G[d,q] = Wk^T @ Q^T first (2.1 GF) and then S^T = x @ G (4.3 GF) replaces
   K-projection (4.3) + scores (4.3). Bonus: G's lhsT is Wk in its NATIVE
   [e,d] layout, and K^T (8MB) is never materialized.

2) out = attnU @ (x @ Wv^T) / sumexp + bv
       = (attnU @ x) @ Wv^T / sumexp + bv
   Z^T[d,q] = x^T-weighted attention (4.3 GF) then out = Z^T.T @ Wv^T
   (2.1 GF) replaces V-projection (4.3) + PV (4.3). The value bias bv
   contributes exactly bv per row (softmax weights sum to 1) and is added in
   the final normalize op. V is never materialized (no DRAM spill).

Matmul layouts (contraction always on the partition dim, zero on-chip
transposes; host supplies xT=[d,s], xN=[s,d], wqT/wvT transposed, wkN native):
  Q^T[e,q]  : lhsT=WqT [d,e-col-tiles], rhs=xT [d,q]      (+bq per-partition)
  G[d,q]    : lhsT=WkN [e,d-slices],    rhs=Q^T [e,q]
  S^T[k,q]  : lhsT=xT  [d,k-slices],    rhs=G   [d,q]
  attnU^T   = exp(S^T/32 + mask_bias[k])  -- ONE fused ScalarE op per tile
              (masked lanes get -30000 -> exp underflows to exact 0; no
              max-subtraction needed: |s/32| <~ 6)
  sumexp    : lhsT=ones [k,2] (M=2),     rhs=attnU^T [k,q] -> [2,q] row,
              then DVE reciprocal + GpSimd partition-broadcast to [128,q];
              the normalize folds into the Z^T psum->SBUF copy (tensor_mul)
  Z^T[d,q]  : lhsT=xN [k,d-slices],      rhs=attnU^T [k,q]  (pre-normalized)
  out[q,dv] : lhsT=Z^T [d,q-slices],     rhs=WvT [d,dv]
  final     : out = psum + bv_bcast  -- one DVE add

All matmuls run in float32r (fp32 bits at bf16-rate: 1 cycle/row for moving
free dim >= 256 vs 4 cycles/row for plain fp32; ~1.6e-4 component error;
HW-verified to accept raw fp32 bit patterns from DRAM directly).

Queue discipline (HWDGE issue is in-order per engine; a compute op waiting on
a semaphore would block DMA issues queued behind it): sync carries W loads +
xN streams + outputs; scalar carries x^T loads + constants (its only compute
is the phase-2 exps); vector does all PSUM->SBUF movement.
"""

from contextlib import ExitStack

import numpy as np

import concourse.bacc as bacc
import concourse.mybir as mybir
import concourse.tile as tile
from concourse.bass_utils import run_bass_kernel_spmd

D = 1024       # model dim = head dim
S = 2048       # sequence length (keys per core)
QL = 1024      # queries per core
N_CORES = 8
SCALE = 1.0 / 32.0   # 1/sqrt(D)
MASK_NEG = -30000.0

F32 = mybir.dt.float32
F32R = mybir.dt.float32r
AF = mybir.ActivationFunctionType
ALU = mybir.AluOpType


def _build_nc():
    nc = bacc.Bacc(None)

    xT = nc.declare_dram_parameter("xT", [D, S], F32R, isOutput=False)[:]
    xN = nc.declare_dram_parameter("xN", [S, D], F32R, isOutput=False)[:]
    wqT = nc.declare_dram_parameter("wqT", [D, D], F32R, isOutput=False)[:]
    wkN = nc.declare_dram_parameter("wkN", [D, D], F32R, isOutput=False)[:]
    wvT = nc.declare_dram_parameter("wvT", [D, D], F32R, isOutput=False)[:]
    bqT = nc.declare_dram_parameter("bqT", [128, 8], F32, isOutput=False)[:]
    mbT = nc.declare_dram_parameter("mbT", [128, 16], F32, isOutput=False)[:]
    bvb = nc.declare_dram_parameter("bvb", [128, D], F32, isOutput=False)[:]
    onesd = nc.declare_dram_parameter("onesd", [128, 2], F32R, isOutput=False)[:]
    out_d = nc.declare_dram_parameter("out", [QL, D], F32, isOutput=True)[:]

    with tile.TileContext(nc) as tc:
        _emit(nc, tc, xT, xN, wqT, wkN, wvT, bqT, mbT, bvb, onesd, out_d)
    nc.finalize()
    return nc


def _emit(nc, tc, xT, xN, wqT, wkN, wvT, bqT, mbT, bvb, onesd, out_d):
    with ExitStack() as ctx:
        consts = ctx.enter_context(tc.tile_pool(name="consts", bufs=1))

        # G[d,q] = Wk^T @ Q^T lives across both phases, 8 d-partition tiles.
        gpool = ctx.enter_context(tc.tile_pool(name="g", bufs=8))
        gt = [gpool.tile([128, QL], F32R, tag="gt", name=f"gt{m}")
              for m in range(8)]
        # xs tiles (S^T lhsT) live in an outer pool so their loads are not
        # gated on the phase-1 pool release -- they stream during G.
        xspool = ctx.enter_context(tc.tile_pool(name="xs", bufs=4))
        # The first attnU^T tiles live outside the phase-2 pool so the first
        # exps are not gated on the phase-1 pool release (PSUM slot recycling
        # would stall the S^T matmul stream at the phase boundary).
        at0pool = ctx.enter_context(tc.tile_pool(name="at0", bufs=4))
        # One PSUM pool for the whole kernel: no pool-release barrier at the
        # phase transition. "ps" (6 banks) serves projections, scores, Z and
        # out; "ps_sum" (2 banks) serves the sumexp accumulators.
        pps = ctx.enter_context(tc.tile_pool(name="ps", bufs=6, space="PSUM"))

        # ---------------- Phase 1: Q^T then G ----------------
        with tc.tile_pool(name="proj", bufs=1) as pp:
            # Q^T [e,q] as 8 e-partition tiles (phase-1 only).
            qt = [pp.tile([128, QL], F32R, tag="qt", bufs=8, name=f"qt{m}")
                  for m in range(8)]

            # wq split by e-column group so the first matmul group only waits
            # on its own 0.5 MB slice.
            wq = []
            for m in range(8):
                w = pp.tile([128, 8, 128], F32R, tag="w", bufs=16,
                            name=f"wq{m}")
                nc.sync.dma_start(
                    out=w,
                    in_=wqT[:, m * 128:(m + 1) * 128]
                    .rearrange("(a p) e -> p a e", p=128))
                wq.append(w)
            xq = []
            xq_dmas = []
            for c in range(2):
                x = pp.tile([128, 8, 512], F32R, tag="x", bufs=2, name=f"xq{c}")
                di = nc.scalar.dma_start(
                    out=x,
                    in_=xT[:, c * 512:(c + 1) * 512]
                    .rearrange("(a p) s -> p a s", p=128))
                xq.append(x)
                xq_dmas.append(di)
            bq_sb = consts.tile([128, 8], F32, tag="bq", name="bq_sb")
            nc.scalar.dma_start(out=bq_sb, in_=bqT)
            mb_sb = consts.tile([128, 16], F32, tag="mb", name="mb_sb")
            nc.scalar.dma_start(out=mb_sb, in_=mbT)
            ones_sb = consts.tile([128, 2], F32R, tag="ones", name="ones_sb")
            nc.scalar.dma_start(out=ones_sb, in_=onesd)
            # Preload the exp table set while the PE is in the projections.
            warm = consts.tile([128, 2], F32, tag="warm", name="warm")
            nc.scalar.activation(warm, ones_sb, AF.Exp)

            # ---- Q^T = WqT.T @ xT[:, 0:1024]  (+ bq per-partition) ----
            for qc in range(2):
                for m in range(8):
                    ps = pps.tile([128, 512], F32, tag="ps", name=f"psq{qc}_{m}")
                    for dk in range(8):
                        nc.tensor.matmul(
                            ps, wq[m][:, dk, :], xq[qc][:, dk, :],
                            start=(dk == 0), stop=(dk == 7))
                    nc.vector.tensor_scalar_add(
                        qt[m][:, qc * 512:(qc + 1) * 512], ps, bq_sb[:, m:m + 1])

            # ---- G[d,q] = WkN.T @ Q^T  (Wk in native [e,d] layout) ----
            # wk tiles are [128e, 1024d] native rows: 4 KB/partition, same
            # slot size as the wq tiles, so they recycle the "w" tag slots.
            wk = []
            for ec in range(8):
                w = pp.tile([128, D], F32R, tag="w", bufs=16, name=f"wk{ec}")
                di = nc.sync.dma_start(out=w, in_=wkN[ec * 128:(ec + 1) * 128, :])
                if ec == 0:
                    # keep the (dep-free, hoistable) wk stream out of the
                    # startup-critical wq/xq DMA window
                    tile.add_dep_helper(
                        di.ins, xq_dmas[1].ins,
                        reason="wk stream after startup loads")
                wk.append(w)
            for dt in range(8):
                for qch in range(2):
                    ps = pps.tile([128, 512], F32, tag="ps",
                                  name=f"psg{dt}_{qch}")
                    for ec in range(8):
                        nc.tensor.matmul(
                            ps,
                            wk[ec][:, dt * 128:(dt + 1) * 128],
                            qt[ec][:, qch * 512:(qch + 1) * 512],
                            start=(ec == 0), stop=(ec == 7))
                    nc.vector.tensor_copy(
                        gt[dt][:, qch * 512:(qch + 1) * 512], ps)

        # ---------------- Phase 2: attention ----------------
        with tc.tile_pool(name="att", bufs=1) as at_p:
            bvb_sb = at_p.tile([128, D], F32, tag="bvb", bufs=1, name="bvb_sb")
            di = nc.scalar.dma_start(out=bvb_sb, in_=bvb)
            tile.add_dep_helper(di.ins, xq_dmas[1].ins,
                                reason="keep hoistable stream out of startup")
            # wv (= Wv^T rows, d-split) resident for the final out-matmul.
            wv = []
            for dt in range(8):
                w = at_p.tile([128, D], F32R, tag="wv", bufs=8, name=f"wv{dt}")
                di = nc.sync.dma_start(out=w, in_=wvT[dt * 128:(dt + 1) * 128, :])
                if dt == 0:
                    tile.add_dep_helper(di.ins, xq_dmas[1].ins,
                                        reason="keep wv stream out of startup")
                wv.append(w)

            # ---- S^T[k,q] = xT.T @ G -> fused mask+exp, both q-chunks ----
            at = [[], []]
            for kt_i in range(16):
                xs = xspool.tile([128, 8, 128], F32R, tag="xs",
                                 name=f"xs{kt_i}")
                di = nc.scalar.dma_start(
                    out=xs,
                    in_=xT[:, kt_i * 128:(kt_i + 1) * 128]
                    .rearrange("(a p) s -> p a s", p=128))
                if kt_i == 0:
                    tile.add_dep_helper(di.ins, xq_dmas[1].ins,
                                        reason="keep xs stream out of startup")
                for qc in range(2):
                    ps = pps.tile([128, 512], F32, tag="ps", name=f"pss{qc}_{kt_i}")
                    for dc in range(8):
                        nc.tensor.matmul(
                            ps,
                            xs[:, dc, :],
                            gt[dc][:, qc * 512:(qc + 1) * 512],
                            start=(dc == 0), stop=(dc == 7))
                    if kt_i < 2:
                        a = at0pool.tile([128, 512], F32R, tag="at0",
                                         name=f"at{qc}_{kt_i}")
                    else:
                        a = at_p.tile([128, 512], F32R, tag="at", bufs=28,
                                      name=f"at{qc}_{kt_i}")
                    nc.scalar.activation(
                        a, ps, AF.Exp,
                        bias=mb_sb[:, kt_i:kt_i + 1], scale=SCALE)
                    at[qc].append(a)

            for qc in range(2):
                # ---- sumexp as a [2,512] row: ones-lhsT matmul (M=2), then
                # reciprocal + GpSimd partition-broadcast; the normalize is
                # folded into the Z^T psum->SBUF copy as a tensor_mul. ----
                srow = pps.tile([2, 512], F32, tag="ps_sum", bufs=2,
                                name=f"srow{qc}")
                for kt_i in range(16):
                    nc.tensor.matmul(
                        srow, ones_sb, at[qc][kt_i],
                        start=(kt_i == 0), stop=(kt_i == 15))
                rrow = at_p.tile([2, 512], F32, tag="rrow", bufs=2,
                                 name=f"rrow{qc}")
                nc.vector.reciprocal(rrow, srow)
                rb = at_p.tile([128, 512], F32, tag="rb", bufs=2,
                               name=f"rb{qc}")
                nc.gpsimd.partition_broadcast(rb, rrow[0:1, :], channels=128)

                # ---- Z^T[d,q] = xN.T @ attnU^T (4 d-tiles per xN pass) ----
                zt = []
                for dth in range(2):
                    pzs = [pps.tile([128, 512], F32, tag="ps", name=f"psz{qc}_{dth}_{j}")
                           for j in range(4)]
                    for kt_i in range(16):
                        xn = at_p.tile([128, 512], F32R, tag="xn", bufs=8,
                                       name=f"xn{qc}_{dth}_{kt_i}")
                        nc.sync.dma_start(
                            out=xn,
                            in_=xN[kt_i * 128:(kt_i + 1) * 128,
                                   dth * 512:(dth + 1) * 512])
                        for j in range(4):
                            nc.tensor.matmul(
                                pzs[j],
                                xn[:, j * 128:(j + 1) * 128],
                                at[qc][kt_i],
                                start=(kt_i == 0), stop=(kt_i == 15))
                    for j in range(4):
                        z = at_p.tile([128, 512], F32R, tag="zt", bufs=8,
                                      name=f"zt{qc}_{dth}_{j}")
                        nc.vector.tensor_mul(z, pzs[j], rb)
                        zt.append(z)

                # ---- out[q,dv] = Z^T.T @ WvT * recip[q] + bv ----
                for qs in range(4):
                    for dvc in range(2):
                        ps = pps.tile([128, 512], F32, tag="ps", name=f"pso{qc}_{qs}_{dvc}")
                        for dt in range(8):
                            nc.tensor.matmul(
                                ps,
                                zt[dt][:, qs * 128:(qs + 1) * 128],
                                wv[dt][:, dvc * 512:(dvc + 1) * 512],
                                start=(dt == 0), stop=(dt == 7))
                        o = at_p.tile([128, 512], F32, tag="o", bufs=4,
                                      name=f"o{qc}_{qs}_{dvc}")
                        nc.vector.tensor_add(
                            o, ps, bvb_sb[:, dvc * 512:(dvc + 1) * 512])
                        row = (qc * 4 + qs) * 128
                        nc.sync.dma_start(
                            out=out_d[row:row + 128, dvc * 512:(dvc + 1) * 512],
                            in_=o)


def _prep_inputs(x, mask, Wq, bq, Wk, bk, Wv, bv):
    x = np.ascontiguousarray(np.asarray(x, dtype=np.float32))
    mask = np.asarray(mask, dtype=bool)
    Wq = np.asarray(Wq, dtype=np.float32)
    bq = np.asarray(bq, dtype=np.float32)
    Wk = np.ascontiguousarray(np.asarray(Wk, dtype=np.float32))
    Wv = np.asarray(Wv, dtype=np.float32)
    bv = np.asarray(bv, dtype=np.float32)
    del bk  # exactly cancelled by softmax shift invariance

    wqT = np.ascontiguousarray(Wq.T)
    wvT = np.ascontiguousarray(Wv.T)
    bqT = np.ascontiguousarray(bq.reshape(8, 128).T)
    bvb = np.ascontiguousarray(np.broadcast_to(bv, (128, D)))
    ones = np.ones((128, 2), dtype=np.float32)

    in_maps = []
    for c in range(N_CORES):
        b, h = divmod(c, 2)
        if h == 0:
            xN_c = x[b]
            mask_c = mask[b]
        else:
            xN_c = np.concatenate([x[b, QL:], x[b, :QL]], axis=0)
            mask_c = np.concatenate([mask[b, QL:], mask[b, :QL]])
        xN_c = np.ascontiguousarray(xN_c)
        xT_c = np.ascontiguousarray(xN_c.T)
        mb = np.where(mask_c, 0.0, MASK_NEG).astype(np.float32)
        mbT = np.ascontiguousarray(mb.reshape(16, 128).T)
        in_maps.append({
            "xT": xT_c, "xN": xN_c, "wqT": wqT, "wkN": Wk, "wvT": wvT,
            "bqT": bqT, "mbT": mbT, "bvb": bvb, "onesd": ones,
        })
    return in_maps


def run(x, mask, Wq, bq, Wk, bk, Wv, bv, trace=False):
    """Build + run; returns (output, BassKernelResults)."""
    in_maps = _prep_inputs(x, mask, Wq, bq, Wk, bk, Wv, bv)
    nc = _build_nc()
    res = run_bass_kernel_spmd(nc, in_maps, list(range(N_CORES)), trace=trace)
    out = np.empty((4, S, D), dtype=np.float32)
    for c in range(N_CORES):
        b, h = divmod(c, 2)
        out[b, h * QL:(h + 1) * QL, :] = res.results[c]["out"]
    return out, res


def kernel(x, mask, Wq, bq, Wk, bk, Wv, bv):
    out, _ = run(x, mask, Wq, bq, Wk, bk, Wv, bv)
    return out



# revision 5
# speedup vs baseline: 1.5620x; 1.5620x over previous
"""Single-head masked attention (B=4, S=2048, D=1024, fp32) on 8 TRN2 NeuronCores.

Sharding: core c handles batch b=c//2, query half h=c%2 (1024 queries).

Two host-side reductions cut the per-core matmul work from the baseline's
928 tile-matmuls to ~560:

1) A-fusion. scores = (x Wq^T + bq)(x_k Wk^T + bk)^T reduces (bk cancels
   under softmax shift invariance) to  x A x_k^T + (bq Wk) x_k^T  with
   A = Wq^T Wk precomputed on the host (f64 accum). On device:
     H[d,q] = A^T xq^T + a_col      (a = bq Wk folded as per-partition add)
     S^T[k,q] = x_k H               (raw scores; no separate Q/K projections)
   This kills the K-projection AND merges Q-projection+score-prep into one
   matmul: 256 tile-matmuls -> 128.

2) Key packing. mask kills ~50% of the 2048 keys; masked keys contribute
   exactly zero (exp(-inf)). The host packs unmasked key rows densely and
   pads to K_pad = 128*K_T (K_T = ceil(max_count/128), same on all cores
   for SPMD); pad lanes get a -30000 exp bias -> exact 0. S^T and Z^T
   shrink from 16 k-tiles to K_T (~9).

Value path (exact as baseline): out^T[dv,q] = Wv^T.T Z^T with
Z^T = x_k^T attnU / sumexp; bv is added per-partition on DVE. Output is
produced TRANSPOSED [D, QL] on device; host transposes after gather.

Matmul layouts (contraction on the partition dim, zero on-chip transposes):
  H[d,q]    : lhsT=A col-tiles [e,(8)128d], rhs=xqT [e,q]   (+a per-part)
  S^T[k,q]  : lhsT=xkT col-tiles [d,(8)128k], rhs=H [d,q]
  attnU^T   = exp(S^T/32 + pad_bias[k])  -- fused ScalarE op per tile
  sumexp    : lhsT=ones [k,2], rhs=attnU^T -> [2,q]; DVE reciprocal +
              GpSimd partition-broadcast; normalize folds into Z copy
  Z^T[d,q]  : lhsT=xkN row-tiles [k,1024d], rhs=attnU^T [k,q]
  out^T[dv,q]: lhsT=wvT row-tiles [d',1024dv], rhs=Z^T [d',q]  (+bv, DVE)

All matmuls in float32r (fp32 bits at bf16 rate).

SBUF discipline: ht (phase 1+2a), xkn (2b) and wv (2c) share one 4KB-slot
pool -- wv allocations recycle ht slots freed when S^T retires. PE issue
order: H -> S^T(qc0) -> sum(qc0) -> S^T(qc1) -> sum(qc1) -> Z(qc0) ->
out(qc0) -> Z(qc1) -> out(qc1), so reciprocal+broadcast and the wv stream
hide under matmul streams.

Queue discipline (HWDGE issue is in-order per engine): sync carries A +
xkT + xkN + wv; scalar carries xq + consts + out writes (its compute:
the exps); vector does all PSUM->SBUF movement + bias adds.
"""

from contextlib import ExitStack

import numpy as np

import concourse.bacc as bacc
import concourse.mybir as mybir
import concourse.tile as tile
from concourse.bass_utils import run_bass_kernel_spmd

D = 1024       # model dim = head dim
QL = 1024      # queries per core
N_CORES = 8
SCALE = 1.0 / 32.0   # 1/sqrt(D)
MASK_NEG = -30000.0

F32 = mybir.dt.float32
F32R = mybir.dt.float32r
AF = mybir.ActivationFunctionType


def _build_nc(K_T):
    K_pad = K_T * 128
    nc = bacc.Bacc(None)

    aD = nc.declare_dram_parameter("aD", [D, D], F32R, isOutput=False)[:]
    xqT = nc.declare_dram_parameter("xqT", [D, QL], F32R, isOutput=False)[:]
    xkT = nc.declare_dram_parameter("xkT", [D, K_pad], F32R, isOutput=False)[:]
    xkN = nc.declare_dram_parameter("xkN", [K_pad, D], F32R, isOutput=False)[:]
    wvT = nc.declare_dram_parameter("wvT", [D, D], F32R, isOutput=False)[:]
    aCol = nc.declare_dram_parameter("aCol", [128, 8], F32, isOutput=False)[:]
    mbT = nc.declare_dram_parameter("mbT", [128, K_T], F32, isOutput=False)[:]
    bvT = nc.declare_dram_parameter("bvT", [128, 8], F32, isOutput=False)[:]
    onesd = nc.declare_dram_parameter("onesd", [128, 2], F32R, isOutput=False)[:]
    out_d = nc.declare_dram_parameter("out", [D, QL], F32, isOutput=True)[:]

    with tile.TileContext(nc) as tc:
        _emit(nc, tc, K_T, aD, xqT, xkT, xkN, wvT, aCol, mbT, bvT, onesd,
              out_d)
    nc.finalize()
    return nc


def _emit(nc, tc, K_T, aD, xqT, xkT, xkN, wvT, aCol, mbT, bvT, onesd, out_d):
    with ExitStack() as ctx:
        consts = ctx.enter_context(tc.tile_pool(name="consts", bufs=1))

        # Shared pool of [128, 1024] f32r slots: ht (8, phase1->S^T),
        # xkN row-tiles (K_T, S^T->Z) and wv row-tiles (8, Z->out). wv
        # recycles ht slots after S^T retires.
        bigpool = ctx.enter_context(tc.tile_pool(name="big", bufs=8 + K_T))
        ht = [bigpool.tile([128, QL], F32R, tag="big", bufs=8 + K_T,
                           name=f"ht{m}")
              for m in range(8)]
        # xkT column tiles; resident through both S^T qc passes.
        xktpool = ctx.enter_context(tc.tile_pool(name="xkt", bufs=K_T))
        # One PSUM pool for the whole kernel (6 banks) + sumexp rows (2).
        pps = ctx.enter_context(tc.tile_pool(name="ps", bufs=6, space="PSUM"))

        # ---------------- Phase 1: H = A^T xq^T + a ----------------
        with tc.tile_pool(name="proj", bufs=1) as pp:
            # A column-tiles: am[m][:, ec, :] = A[ec*128:(ec+1)*128,
            # m*128:(m+1)*128]; only am[0] gates the first matmul group.
            am = []
            for m in range(8):
                w = pp.tile([128, 8, 128], F32R, tag="am", bufs=8,
                            name=f"am{m}")
                nc.sync.dma_start(
                    out=w,
                    in_=aD[:, m * 128:(m + 1) * 128]
                    .rearrange("(a p) c -> p a c", p=128))
                am.append(w)
            # xq tiles split by (ec, qc); the startup-critical qc=0 set
            # loads first on scalar, in parallel with A on sync.
            xq = [[None] * 2 for _ in range(8)]
            xq_last_dma = None
            for qc in range(2):
                for ec in range(8):
                    t = pp.tile([128, 512], F32R, tag="xq", bufs=16,
                                name=f"xq{ec}_{qc}")
                    di = nc.scalar.dma_start(
                        out=t,
                        in_=xqT[ec * 128:(ec + 1) * 128,
                                qc * 512:(qc + 1) * 512])
                    xq[ec][qc] = t
                    xq_last_dma = di
            aCol_sb = consts.tile([128, 8], F32, tag="aCol", name="aCol_sb")
            nc.scalar.dma_start(out=aCol_sb, in_=aCol)
            mb_sb = consts.tile([128, K_T], F32, tag="mb", name="mb_sb")
            nc.scalar.dma_start(out=mb_sb, in_=mbT)
            bv_sb = consts.tile([128, 8], F32, tag="bv", name="bv_sb")
            nc.scalar.dma_start(out=bv_sb, in_=bvT)
            ones_sb = consts.tile([128, 2], F32R, tag="ones", name="ones_sb")
            nc.scalar.dma_start(out=ones_sb, in_=onesd)
            # Preload the exp table set while the PE is in phase 1.
            warm = consts.tile([128, 2], F32, tag="warm", name="warm")
            nc.scalar.activation(warm, ones_sb, AF.Exp)

            # xkT column tiles are dep-free -> keep their stream out of the
            # startup-critical am/xq DMA window.
            xkt = []
            for kt in range(K_T):
                w = xktpool.tile([128, 8, 128], F32R, tag="xkt", bufs=K_T,
                                 name=f"xkt{kt}")
                di = nc.sync.dma_start(
                    out=w,
                    in_=xkT[:, kt * 128:(kt + 1) * 128]
                    .rearrange("(a p) c -> p a c", p=128))
                if kt == 0:
                    tile.add_dep_helper(
                        di.ins, xq_last_dma.ins,
                        reason="xkt stream after startup loads")
                xkt.append(w)

            # ---- H groups: for qc, m: accumulate over ec ----
            for qc in range(2):
                for m in range(8):
                    ps = pps.tile([128, 512], F32, tag="ps",
                                  name=f"psh{qc}_{m}")
                    for ec in range(8):
                        nc.tensor.matmul(
                            ps, am[m][:, ec, :], xq[ec][qc],
                            start=(ec == 0), stop=(ec == 7))
                    nc.vector.tensor_scalar_add(
                        ht[m][:, qc * 512:(qc + 1) * 512], ps,
                        aCol_sb[:, m:m + 1])

        # ---------------- Phase 2: scores, softmax, values ----------------
        with tc.tile_pool(name="att", bufs=1) as at_p:
            # xkN row-tiles (resident through Z^T), then wv row-tiles;
            # both stream on sync behind xkt, recycling bigpool slots.
            xkn = []
            for kt in range(K_T):
                w = bigpool.tile([128, D], F32R, tag="big", bufs=8 + K_T,
                                 name=f"xkn{kt}")
                di = nc.sync.dma_start(
                    out=w, in_=xkN[kt * 128:(kt + 1) * 128, :])
                if kt == 0:
                    tile.add_dep_helper(di.ins, xq_last_dma.ins,
                                        reason="xkn stream out of startup")
                xkn.append(w)
            wv = []
            for dp in range(8):
                w = bigpool.tile([128, D], F32R, tag="big", bufs=8 + K_T,
                                 name=f"wv{dp}")
                nc.sync.dma_start(out=w, in_=wvT[dp * 128:(dp + 1) * 128, :])
                wv.append(w)

            # ---- S^T[k,q] = xkT.T @ H -> fused pad-bias+exp; sumexp ----
            at = [[None] * K_T for _ in range(2)]
            rbs = []
            for qc in range(2):
                for kt in range(K_T):
                    ps = pps.tile([128, 512], F32, tag="ps",
                                  name=f"pss{qc}_{kt}")
                    for dc in range(8):
                        nc.tensor.matmul(
                            ps, xkt[kt][:, dc, :],
                            ht[dc][:, qc * 512:(qc + 1) * 512],
                            start=(dc == 0), stop=(dc == 7))
                    a = at_p.tile([128, 512], F32R, tag="at", bufs=2 * K_T,
                                  name=f"at{qc}_{kt}")
                    nc.scalar.activation(
                        a, ps, AF.Exp,
                        bias=mb_sb[:, kt:kt + 1], scale=SCALE)
                    at[qc][kt] = a

                # sumexp -> reciprocal -> partition broadcast; overlaps the
                # next S^T pass / Z stream on PE.
                srow = pps.tile([2, 512], F32, tag="ps_sum", bufs=2,
                                name=f"srow{qc}")
                for kt in range(K_T):
                    nc.tensor.matmul(
                        srow, ones_sb, at[qc][kt],
                        start=(kt == 0), stop=(kt == K_T - 1))
                rrow = at_p.tile([2, 512], F32, tag="rrow", bufs=2,
                                 name=f"rrow{qc}")
                nc.vector.reciprocal(rrow[0:1, :], srow[0:1, :])
                rb = at_p.tile([128, 512], F32, tag="rb", bufs=2,
                               name=f"rb{qc}")
                nc.gpsimd.partition_broadcast(rb, rrow[0:1, :], channels=128)
                rbs.append(rb)

            # ---- per qc: Z^T then out^T (interleaved passes) ----
            for qc in range(2):
                zt = []
                for dt in range(8):
                    ps = pps.tile([128, 512], F32, tag="ps",
                                  name=f"psz{qc}_{dt}")
                    for kt in range(K_T):
                        nc.tensor.matmul(
                            ps, xkn[kt][:, dt * 128:(dt + 1) * 128],
                            at[qc][kt],
                            start=(kt == 0), stop=(kt == K_T - 1))
                    z = at_p.tile([128, 512], F32R, tag="zt", bufs=12,
                                  name=f"zt{qc}_{dt}")
                    nc.vector.tensor_mul(z, ps, rbs[qc])
                    zt.append(z)

                for dvt in range(8):
                    ps = pps.tile([128, 512], F32, tag="ps",
                                  name=f"pso{qc}_{dvt}")
                    for dp in range(8):
                        nc.tensor.matmul(
                            ps, wv[dp][:, dvt * 128:(dvt + 1) * 128],
                            zt[dp],
                            start=(dp == 0), stop=(dp == 7))
                    o = at_p.tile([128, 512], F32, tag="o", bufs=4,
                                  name=f"o{qc}_{dvt}")
                    nc.vector.tensor_scalar_add(
                        o, ps, bv_sb[:, dvt:dvt + 1])
                    nc.scalar.dma_start(
                        out=out_d[dvt * 128:(dvt + 1) * 128,
                                  qc * 512:(qc + 1) * 512],
                        in_=o)


def _prep_inputs(x, mask, Wq, bq, Wk, bk, Wv, bv):
    x = np.ascontiguousarray(np.asarray(x, dtype=np.float32))
    mask = np.asarray(mask, dtype=bool)
    Wq = np.asarray(Wq, dtype=np.float64)
    bq = np.asarray(bq, dtype=np.float64)
    Wk = np.asarray(Wk, dtype=np.float64)
    Wv = np.asarray(Wv, dtype=np.float32)
    bv = np.asarray(bv, dtype=np.float32)
    del bk  # exactly cancelled by softmax shift invariance

    A = np.ascontiguousarray((Wq.T @ Wk).astype(np.float32))
    a_vec = (bq @ Wk).astype(np.float32)
    wvT = np.ascontiguousarray(Wv.T)
    aColT = np.ascontiguousarray(a_vec.reshape(8, 128).T)
    bvT = np.ascontiguousarray(bv.reshape(8, 128).T)
    ones = np.ones((128, 2), dtype=np.float32)

    counts = mask.sum(axis=1)
    K_T = int(np.ceil(counts.max() / 128))
    K_pad = K_T * 128

    in_maps = []
    for c in range(N_CORES):
        b, h = divmod(c, 2)
        sel = np.where(mask[b])[0]
        K = len(sel)
        xk = np.zeros((K_pad, D), dtype=np.float32)
        xk[:K] = x[b, sel]
        mb = np.zeros(K_pad, dtype=np.float32)
        mb[K:] = MASK_NEG
        mbT = np.ascontiguousarray(mb.reshape(K_T, 128).T)
        xq_c = np.ascontiguousarray(x[b, h * QL:(h + 1) * QL].T)
        in_maps.append({
            "aD": A, "xqT": xq_c,
            "xkT": np.ascontiguousarray(xk.T),
            "xkN": xk, "wvT": wvT,
            "aCol": aColT, "mbT": mbT, "bvT": bvT, "onesd": ones,
        })
    return in_maps, K_T


def run(x, mask, Wq, bq, Wk, bk, Wv, bv, trace=False):
    """Build + run; returns (output, BassKernelResults)."""
    in_maps, K_T = _prep_inputs(x, mask, Wq, bq, Wk, bk, Wv, bv)
    nc = _build_nc(K_T)
    res = run_bass_kernel_spmd(nc, in_maps, list(range(N_CORES)), trace=trace)
    out = np.empty((4, 2048, D), dtype=np.float32)
    for c in range(N_CORES):
        b, h = divmod(c, 2)
        out[b, h * QL:(h + 1) * QL, :] = res.results[c]["out"].T
    return out, res


def kernel(x, mask, Wq, bq, Wk, bk, Wv, bv):
    out, _ = run(x, mask, Wq, bq, Wk, bk, Wv, bv)
    return out


# revision 6
# speedup vs baseline: 1.7488x; 1.1196x over previous
"""Single-head masked attention (B=4, S=2048, D=1024, fp32) on 8 TRN2 NeuronCores.

Sharding: core c handles batch b=c//2, query half h=c%2 (1024 queries).

Three reductions versus a direct implementation:

1) A-fusion. scores = (x Wq^T + bq)(x_k Wk^T + bk)^T reduces (bk cancels
   under softmax shift invariance) to  x A x_k^T + (bq Wk) x_k^T  with
   A = Wq^T Wk precomputed on the host (f64 accum). On device:
     H[d,q] = A^T xq^T + a_col      (a = bq Wk folded as per-partition add)
     S^T[k,q] = x_k H               (raw scores; no separate Q/K projections)

2) Key packing. mask kills ~50% of the 2048 keys; masked keys contribute
   exactly zero (exp(-inf)). The host packs unmasked key rows densely and
   pads to K_pad = 128*K_T (K_T = ceil(max_count/128), same on all cores
   for SPMD); pad lanes get a -30000 exp bias -> exact 0. S^T and Z^T
   shrink from 16 k-tiles to K_T (~9).

3) All matmul operands in bf16 (PSUM accumulation stays f32). Same PE rate
   as float32r (1 cycle/row) but half the HBM traffic -- phase 1 was DMA-
   bandwidth-bound in f32r. Measured end-to-end error ~6e-3 vs the 2e-2
   gate (softmax averaging washes out the per-element quantization).

Value path: out^T[dv,q] = Wv^T.T Z^T with Z^T = x_k^T attnU / sumexp; bv
is added per-partition on DVE. Output is produced TRANSPOSED [D, QL] on
device; host transposes after gather.

Matmul layouts (contraction on the partition dim, zero on-chip transposes;
A and xkT are pre-tiled on the host so every DMA is 128x2KB contiguous):
  H[d,q]    : lhsT=A col-tiles [e,(8)128d], rhs=xqT [e,q]   (+a per-part)
  S^T[k,q]  : lhsT=xkT col-tiles [d,(8)128k], rhs=H [d,q]
  attnU^T   = exp(S^T/32 + pad_bias[k])  -- fused ScalarE op per tile
  sumexp    : lhsT=ones [k,2], rhs=attnU^T -> [2,q]; DVE reciprocal +
              GpSimd partition-broadcast; normalize folds into Z copy
  Z^T[d,q]  : lhsT=xkN row-tiles [k,1024d], rhs=attnU^T [k,q]
  out^T[dv,q]: lhsT=wvT row-tiles [d',1024dv], rhs=Z^T [d',q]  (+bv, DVE)

PE issue order: H -> S^T(qc0) -> sum(qc0) -> S^T(qc1) -> sum(qc1) ->
Z(qc0) -> out(qc0) -> Z(qc1) -> out(qc1), so reciprocal+broadcast and the
wv stream hide under matmul streams.

Queue discipline (HWDGE issue is in-order per engine): sync carries A +
xkT + xkN + wv + qc1 out writes; scalar carries xq + consts + qc0 out
writes (its compute: the exps); vector does all PSUM->SBUF movement +
bias adds.
"""

from contextlib import ExitStack

import ml_dtypes
import numpy as np

import concourse.bacc as bacc
import concourse.mybir as mybir
import concourse.tile as tile
from concourse.bass_utils import run_bass_kernel_spmd

D = 1024       # model dim = head dim
QL = 1024      # queries per core
N_CORES = 8
SCALE = 1.0 / 32.0   # 1/sqrt(D)
MASK_NEG = -30000.0

F32 = mybir.dt.float32
BF16 = mybir.dt.bfloat16
AF = mybir.ActivationFunctionType
NP_BF16 = ml_dtypes.bfloat16


def _build_nc(K_T):
    K_pad = K_T * 128
    nc = bacc.Bacc(None)

    # aP[m] / xkP[kt] are host-pre-tiled so each [128, 8, 128] lhsT tile
    # is a contiguous [128 x 2KB] DMA.
    aP = nc.declare_dram_parameter("aP", [8, 128, D], BF16, isOutput=False)[:]
    xqT = nc.declare_dram_parameter("xqT", [D, QL], BF16, isOutput=False)[:]
    xkP = nc.declare_dram_parameter("xkP", [K_T, 128, D], BF16,
                                    isOutput=False)[:]
    xkN = nc.declare_dram_parameter("xkN", [K_pad, D], BF16, isOutput=False)[:]
    wvT = nc.declare_dram_parameter("wvT", [D, D], BF16, isOutput=False)[:]
    aCol = nc.declare_dram_parameter("aCol", [128, 8], F32, isOutput=False)[:]
    mbT = nc.declare_dram_parameter("mbT", [128, K_T], F32, isOutput=False)[:]
    bvT = nc.declare_dram_parameter("bvT", [128, 8], F32, isOutput=False)[:]
    onesd = nc.declare_dram_parameter("onesd", [128, 2], BF16,
                                      isOutput=False)[:]
    out_d = nc.declare_dram_parameter("out", [D, QL], F32, isOutput=True)[:]

    with tile.TileContext(nc) as tc:
        _emit(nc, tc, K_T, aP, xqT, xkP, xkN, wvT, aCol, mbT, bvT, onesd,
              out_d)
    nc.finalize()
    return nc


def _emit(nc, tc, K_T, aP, xqT, xkP, xkN, wvT, aCol, mbT, bvT, onesd, out_d):
    with ExitStack() as ctx:
        consts = ctx.enter_context(tc.tile_pool(name="consts", bufs=1))

        # H row-tiles [128, 1024], live phase 1 -> end of S^T.
        hpool = ctx.enter_context(tc.tile_pool(name="h", bufs=8))
        ht = [hpool.tile([128, QL], BF16, tag="ht", name=f"ht{m}")
              for m in range(8)]
        # xkT column tiles; resident through both S^T qc passes.
        xktpool = ctx.enter_context(tc.tile_pool(name="xkt", bufs=K_T))
        # attnU^T tiles [k,q] live from S^T through Z^T.
        atpool = ctx.enter_context(tc.tile_pool(name="at", bufs=2 * K_T))
        # xkN row-tiles (S^T prefetch -> Z) and wv row-tiles (-> out).
        xknpool = ctx.enter_context(tc.tile_pool(name="xkn", bufs=K_T))
        wvpool = ctx.enter_context(tc.tile_pool(name="wv", bufs=8))
        # One PSUM pool for the whole kernel (6 banks) + sumexp rows (2).
        pps = ctx.enter_context(tc.tile_pool(name="ps", bufs=6, space="PSUM"))

        # ---------------- Phase 1: H = A^T xq^T + a ----------------
        with tc.tile_pool(name="proj", bufs=1) as pp:
            # A column-tiles: am[m][:, ec, :] = A[ec*128:(ec+1)*128,
            # m*128:(m+1)*128]; only am[0] gates the first matmul group.
            am = []
            for m in range(8):
                w = pp.tile([128, 8, 128], BF16, tag="am", bufs=8,
                            name=f"am{m}")
                nc.sync.dma_start(
                    out=w, in_=aP[m].rearrange("p (a c) -> p a c", a=8))
                am.append(w)
            # xq tiles split by (ec, qc); the startup-critical qc=0 set
            # loads first on scalar, in parallel with A on sync.
            xq = [[None] * 2 for _ in range(8)]
            xq_last_dma = None
            for qc in range(2):
                for ec in range(8):
                    t = pp.tile([128, 512], BF16, tag="xq", bufs=16,
                                name=f"xq{ec}_{qc}")
                    di = nc.scalar.dma_start(
                        out=t,
                        in_=xqT[ec * 128:(ec + 1) * 128,
                                qc * 512:(qc + 1) * 512])
                    xq[ec][qc] = t
                    xq_last_dma = di
            aCol_sb = consts.tile([128, 8], F32, tag="aCol", name="aCol_sb")
            nc.scalar.dma_start(out=aCol_sb, in_=aCol)
            mb_sb = consts.tile([128, K_T], F32, tag="mb", name="mb_sb")
            nc.scalar.dma_start(out=mb_sb, in_=mbT)
            bv_sb = consts.tile([128, 8], F32, tag="bv", name="bv_sb")
            nc.scalar.dma_start(out=bv_sb, in_=bvT)
            ones_sb = consts.tile([128, 2], BF16, tag="ones", name="ones_sb")
            nc.scalar.dma_start(out=ones_sb, in_=onesd)
            # Preload the exp table set while the PE is in phase 1.
            warm = consts.tile([128, 2], F32, tag="warm", name="warm")
            nc.scalar.activation(warm, ones_sb, AF.Exp)

            # xkT column tiles are dep-free -> keep their stream out of the
            # startup-critical am/xq DMA window.
            xkt = []
            for kt in range(K_T):
                w = xktpool.tile([128, 8, 128], BF16, tag="xkt", bufs=K_T,
                                 name=f"xkt{kt}")
                di = nc.sync.dma_start(
                    out=w, in_=xkP[kt].rearrange("p (a c) -> p a c", a=8))
                if kt == 0:
                    tile.add_dep_helper(
                        di.ins, xq_last_dma.ins,
                        reason="xkt stream after startup loads")
                xkt.append(w)

            # ---- H groups: for qc, m: accumulate over ec ----
            for qc in range(2):
                for m in range(8):
                    ps = pps.tile([128, 512], F32, tag="ps",
                                  name=f"psh{qc}_{m}")
                    for ec in range(8):
                        nc.tensor.matmul(
                            ps, am[m][:, ec, :], xq[ec][qc],
                            start=(ec == 0), stop=(ec == 7))
                    nc.vector.tensor_scalar_add(
                        ht[m][:, qc * 512:(qc + 1) * 512], ps,
                        aCol_sb[:, m:m + 1])

        # ---------------- Phase 2: scores, softmax, values ----------------
        with tc.tile_pool(name="att", bufs=1) as at_p:
            # xkN row-tiles (resident through Z^T), then wv row-tiles;
            # both stream on sync behind xkt.
            xkn = []
            for kt in range(K_T):
                w = xknpool.tile([128, D], BF16, tag="xkn", bufs=K_T,
                                 name=f"xkn{kt}")
                di = nc.sync.dma_start(
                    out=w, in_=xkN[kt * 128:(kt + 1) * 128, :])
                if kt == 0:
                    tile.add_dep_helper(di.ins, xq_last_dma.ins,
                                        reason="xkn stream out of startup")
                xkn.append(w)
            wv = []
            for dp in range(8):
                w = wvpool.tile([128, D], BF16, tag="wv", bufs=8,
                                name=f"wv{dp}")
                nc.sync.dma_start(out=w, in_=wvT[dp * 128:(dp + 1) * 128, :])
                wv.append(w)

            # ---- S^T[k,q] = xkT.T @ H -> fused pad-bias+exp; sumexp ----
            at = [[None] * K_T for _ in range(2)]
            rbs = []
            for qc in range(2):
                for kt in range(K_T):
                    ps = pps.tile([128, 512], F32, tag="ps",
                                  name=f"pss{qc}_{kt}")
                    for dc in range(8):
                        nc.tensor.matmul(
                            ps, xkt[kt][:, dc, :],
                            ht[dc][:, qc * 512:(qc + 1) * 512],
                            start=(dc == 0), stop=(dc == 7))
                    a = at_p.tile([128, 512], BF16, tag="at", bufs=2 * K_T,
                                  name=f"at{qc}_{kt}")
                    nc.scalar.activation(
                        a, ps, AF.Exp,
                        bias=mb_sb[:, kt:kt + 1], scale=SCALE)
                    at[qc][kt] = a

                # sumexp -> reciprocal -> partition broadcast; overlaps the
                # next S^T pass / Z stream on PE.
                srow = pps.tile([2, 512], F32, tag="ps_sum", bufs=2,
                                name=f"srow{qc}")
                for kt in range(K_T):
                    nc.tensor.matmul(
                        srow, ones_sb, at[qc][kt],
                        start=(kt == 0), stop=(kt == K_T - 1))
                rrow = at_p.tile([2, 512], F32, tag="rrow", bufs=2,
                                 name=f"rrow{qc}")
                nc.vector.reciprocal(rrow[0:1, :], srow[0:1, :])
                rb = at_p.tile([128, 512], F32, tag="rb", bufs=2,
                               name=f"rb{qc}")
                nc.gpsimd.partition_broadcast(rb, rrow[0:1, :], channels=128)
                rbs.append(rb)

            # ---- per qc: Z^T then out^T (interleaved passes) ----
            for qc in range(2):
                zt = []
                for dt in range(8):
                    ps = pps.tile([128, 512], F32, tag="ps",
                                  name=f"psz{qc}_{dt}")
                    for kt in range(K_T):
                        nc.tensor.matmul(
                            ps, xkn[kt][:, dt * 128:(dt + 1) * 128],
                            at[qc][kt],
                            start=(kt == 0), stop=(kt == K_T - 1))
                    z = at_p.tile([128, 512], BF16, tag="zt", bufs=12,
                                  name=f"zt{qc}_{dt}")
                    nc.vector.tensor_mul(z, ps, rbs[qc])
                    zt.append(z)

                for dvt in range(8):
                    ps = pps.tile([128, 512], F32, tag="ps",
                                  name=f"pso{qc}_{dvt}")
                    for dp in range(8):
                        nc.tensor.matmul(
                            ps, wv[dp][:, dvt * 128:(dvt + 1) * 128],
                            zt[dp],
                            start=(dp == 0), stop=(dp == 7))
                    o = at_p.tile([128, 512], F32, tag="o", bufs=4,
                                  name=f"o{qc}_{dvt}")
                    nc.vector.tensor_scalar_add(
                        o, ps, bv_sb[:, dvt:dvt + 1])
                    weng = nc.scalar if qc == 0 else nc.sync
                    weng.dma_start(
                        out=out_d[dvt * 128:(dvt + 1) * 128,
                                  qc * 512:(qc + 1) * 512],
                        in_=o)


def _prep_inputs(x, mask, Wq, bq, Wk, bk, Wv, bv):
    x = np.ascontiguousarray(np.asarray(x, dtype=np.float32))
    mask = np.asarray(mask, dtype=bool)
    Wq = np.asarray(Wq, dtype=np.float64)
    bq = np.asarray(bq, dtype=np.float64)
    Wk = np.asarray(Wk, dtype=np.float64)
    Wv = np.asarray(Wv, dtype=np.float32)
    bv = np.asarray(bv, dtype=np.float32)
    del bk  # exactly cancelled by softmax shift invariance

    A = (Wq.T @ Wk).astype(np.float32)
    a_vec = (bq @ Wk).astype(np.float32)
    # aP[m, p, a*128+c] = A[a*128+p, m*128+c]
    aP = np.ascontiguousarray(
        A.reshape(8, 128, 8, 128).transpose(2, 1, 0, 3)
        .reshape(8, 128, D).astype(NP_BF16))
    wvT = np.ascontiguousarray(Wv.T.astype(NP_BF16))
    aColT = np.ascontiguousarray(a_vec.reshape(8, 128).T)
    bvT = np.ascontiguousarray(bv.reshape(8, 128).T)
    ones = np.ones((128, 2), dtype=NP_BF16)

    counts = mask.sum(axis=1)
    K_T = int(np.ceil(counts.max() / 128))
    K_pad = K_T * 128

    in_maps = []
    for c in range(N_CORES):
        b, h = divmod(c, 2)
        sel = np.where(mask[b])[0]
        K = len(sel)
        xk = np.zeros((K_pad, D), dtype=np.float32)
        xk[:K] = x[b, sel]
        mb = np.zeros(K_pad, dtype=np.float32)
        mb[K:] = MASK_NEG
        mbT = np.ascontiguousarray(mb.reshape(K_T, 128).T)
        xq_c = np.ascontiguousarray(
            x[b, h * QL:(h + 1) * QL].T.astype(NP_BF16))
        # xkP[kt, p, dc*128+c] = xk[kt*128+c, dc*128+p]
        xkP = np.ascontiguousarray(
            xk.reshape(K_T, 128, 8, 128).transpose(0, 3, 2, 1)
            .reshape(K_T, 128, D).astype(NP_BF16))
        in_maps.append({
            "aP": aP, "xqT": xq_c, "xkP": xkP,
            "xkN": np.ascontiguousarray(xk.astype(NP_BF16)),
            "wvT": wvT,
            "aCol": aColT, "mbT": mbT, "bvT": bvT, "onesd": ones,
        })
    return in_maps, K_T


def run(x, mask, Wq, bq, Wk, bk, Wv, bv, trace=False):
    """Build + run; returns (output, BassKernelResults)."""
    in_maps, K_T = _prep_inputs(x, mask, Wq, bq, Wk, bk, Wv, bv)
    nc = _build_nc(K_T)
    res = run_bass_kernel_spmd(nc, in_maps, list(range(N_CORES)), trace=trace)
    out = np.empty((4, 2048, D), dtype=np.float32)
    for c in range(N_CORES):
        b, h = divmod(c, 2)
        out[b, h * QL:(h + 1) * QL, :] = res.results[c]["out"].T
    return out, res


def kernel(x, mask, Wq, bq, Wk, bk, Wv, bv):
    out, _ = run(x, mask, Wq, bq, Wk, bk, Wv, bv)
    return out


# revision 9
# speedup vs baseline: 1.7826x; 1.0194x over previous
"""Single-head masked attention (B=4, S=2048, D=1024, fp32) on 8 TRN2 NeuronCores.

Sharding: core c handles batch b=c//2, query half h=c%2 (1024 queries).

Three reductions versus a direct implementation:

1) A-fusion. scores = (x Wq^T + bq)(x_k Wk^T + bk)^T reduces (bk cancels
   under softmax shift invariance) to  x A x_k^T + (bq Wk) x_k^T  with
   A = Wq^T Wk precomputed on the host (f64 accum). On device:
     H[d,q] = A^T xq^T + a_col      (a = bq Wk folded as per-partition add)
     S^T[k,q] = x_k H               (raw scores; no separate Q/K projections)

2) Key packing. mask kills ~50% of the 2048 keys; masked keys contribute
   exactly zero (exp(-inf)). The host packs unmasked key rows densely and
   pads to K_pad = 128*K_T (K_T = ceil(max_count/128), same on all cores
   for SPMD); pad lanes get a -30000 exp bias -> exact 0. S^T and Z^T
   shrink from 16 k-tiles to K_T (~9).

3) All matmul operands in bf16 (PSUM accumulation stays f32). Same PE rate
   as float32r (1 cycle/row) but half the HBM traffic -- phase 1 was DMA-
   bandwidth-bound in f32r. Measured end-to-end error ~6e-3 vs the 2e-2
   gate (softmax averaging washes out the per-element quantization).

Value path: out^T[dv,q] = Wv^T.T Z^T with Z^T = x_k^T attnU / sumexp; bv
is added per-partition on DVE. Output is produced TRANSPOSED [D, QL] on
device; host transposes after gather.

Matmul layouts (contraction on the partition dim, zero on-chip transposes;
A and xkT are pre-tiled on the host so every DMA is 128x2KB contiguous):
  H[d,q]    : lhsT=A col-tiles [e,(8)128d], rhs=xqT [e,q]   (+a per-part)
  S^T[k,q]  : lhsT=xkT col-tiles [d,(8)128k], rhs=H [d,q]
  attnU^T   = exp(S^T/32 + pad_bias[k])  -- fused ScalarE op per tile
  sumexp    : lhsT=ones [k,2], rhs=attnU^T -> [2,q]; DVE reciprocal +
              GpSimd partition-broadcast; normalize folds into Z copy
  Z^T[d,q]  : lhsT=xkN row-tiles [k,1024d], rhs=attnU^T [k,q]
  out^T[dv,q]: lhsT=wvT row-tiles [d',1024dv], rhs=Z^T [d',q]  (+bv, DVE)

PE issue order: H -> S^T(qc0) -> sum(qc0) -> S^T(qc1) -> sum(qc1) ->
Z(qc0) -> out(qc0) -> Z(qc1) -> out(qc1), so reciprocal+broadcast and the
wv stream hide under matmul streams.

Queue discipline (HWDGE issue is in-order per engine): sync carries A +
xkT + xkN + wv + qc1 out writes; scalar carries xq + consts + qc0 out
writes (its compute: the exps); vector does all PSUM->SBUF movement +
bias adds.
"""

from contextlib import ExitStack

import ml_dtypes
import numpy as np

import concourse.bacc as bacc
import concourse.mybir as mybir
import concourse.tile as tile
from concourse.bass_utils import run_bass_kernel_spmd

D = 1024       # model dim = head dim
QL = 1024      # queries per core
N_CORES = 8
SCALE = 1.0 / 32.0   # 1/sqrt(D)
MASK_NEG = -30000.0

F32 = mybir.dt.float32
BF16 = mybir.dt.bfloat16
AF = mybir.ActivationFunctionType
NP_BF16 = ml_dtypes.bfloat16


def _build_nc(K_T):
    K_pad = K_T * 128
    nc = bacc.Bacc(None)

    # aP[m] / xkP[kt] are host-pre-tiled so each [128, 8, 128] lhsT tile
    # is a contiguous [128 x 2KB] DMA.
    aP = nc.declare_dram_parameter("aP", [8, 128, D], BF16, isOutput=False)[:]
    xqT = nc.declare_dram_parameter("xqT", [D, QL], BF16, isOutput=False)[:]
    xkP = nc.declare_dram_parameter("xkP", [K_T, 128, D], BF16,
                                    isOutput=False)[:]
    xkN = nc.declare_dram_parameter("xkN", [K_pad, D], BF16, isOutput=False)[:]
    wvT = nc.declare_dram_parameter("wvT", [D, D], BF16, isOutput=False)[:]
    aCol = nc.declare_dram_parameter("aCol", [128, 8], F32, isOutput=False)[:]
    mbT = nc.declare_dram_parameter("mbT", [128, K_T], F32, isOutput=False)[:]
    bvT = nc.declare_dram_parameter("bvT", [128, 8], F32, isOutput=False)[:]
    onesd = nc.declare_dram_parameter("onesd", [128, 2], BF16,
                                      isOutput=False)[:]
    out_d = nc.declare_dram_parameter("out", [D, QL], F32, isOutput=True)[:]

    with tile.TileContext(nc) as tc:
        _emit(nc, tc, K_T, aP, xqT, xkP, xkN, wvT, aCol, mbT, bvT, onesd,
              out_d)
    nc.finalize()
    return nc


def _emit(nc, tc, K_T, aP, xqT, xkP, xkN, wvT, aCol, mbT, bvT, onesd, out_d):
    with ExitStack() as ctx:
        consts = ctx.enter_context(tc.tile_pool(name="consts", bufs=1))

        # H row-tiles [128, 1024], live phase 1 -> end of S^T.
        hpool = ctx.enter_context(tc.tile_pool(name="h", bufs=8))
        ht = [hpool.tile([128, QL], BF16, tag="ht", name=f"ht{m}")
              for m in range(8)]
        # xkT column tiles; resident through both S^T qc passes.
        xktpool = ctx.enter_context(tc.tile_pool(name="xkt", bufs=K_T))
        # attnU^T tiles [k,q] live from S^T through Z^T.
        atpool = ctx.enter_context(tc.tile_pool(name="at", bufs=2 * K_T))
        # xkN row-tiles (S^T prefetch -> Z) and wv row-tiles (-> out).
        xknpool = ctx.enter_context(tc.tile_pool(name="xkn", bufs=K_T))
        wvpool = ctx.enter_context(tc.tile_pool(name="wv", bufs=8))
        # One PSUM pool for the whole kernel (6 banks) + sumexp rows (2).
        pps = ctx.enter_context(tc.tile_pool(name="ps", bufs=6, space="PSUM"))

        # ---------------- Phase 1: H = A^T xq^T + a ----------------
        with tc.tile_pool(name="proj", bufs=1) as pp:
            # A column-tiles: am[m][:, ec, :] = A[ec*128:(ec+1)*128,
            # m*128:(m+1)*128]; only am[0] gates the first matmul group.
            # xq as 4 combined tiles [128, 4ec, 512] -- few DMA issues, and
            # the startup-critical qc=0 pair is split across both queues so
            # it lands in ~2 transfers.
            am = [pp.tile([128, 8, 128], BF16, tag="am", bufs=8,
                          name=f"am{m}") for m in range(8)]
            xq4 = [[None] * 2 for _ in range(2)]  # [g][qc], ec = 4g..4g+3
            for g in range(2):
                for qc in range(2):
                    xq4[g][qc] = pp.tile([128, 4, 512], BF16, tag="xq",
                                         bufs=4, name=f"xq{g}_{qc}")

            def ld_xq(eng, g, qc):
                return eng.dma_start(
                    out=xq4[g][qc],
                    in_=xqT[g * 512:(g + 1) * 512,
                            qc * 512:(qc + 1) * 512]
                    .rearrange("(a p) q -> p a q", p=128))

            nc.sync.dma_start(
                out=am[0], in_=aP[0].rearrange("p (a c) -> p a c", a=8))
            ld_xq(nc.scalar, 0, 0)
            ld_xq(nc.sync, 1, 0)
            for m in range(1, 8):
                nc.sync.dma_start(
                    out=am[m], in_=aP[m].rearrange("p (a c) -> p a c", a=8))
            ld_xq(nc.scalar, 0, 1)
            xq_last_dma = ld_xq(nc.scalar, 1, 1)

            def xq_slice(ec, qc):
                return xq4[ec // 4][qc][:, ec % 4, :]
            aCol_sb = consts.tile([128, 8], F32, tag="aCol", name="aCol_sb")
            nc.scalar.dma_start(out=aCol_sb, in_=aCol)
            mb_sb = consts.tile([128, K_T], F32, tag="mb", name="mb_sb")
            nc.scalar.dma_start(out=mb_sb, in_=mbT)
            bv_sb = consts.tile([128, 8], F32, tag="bv", name="bv_sb")
            nc.scalar.dma_start(out=bv_sb, in_=bvT)
            ones_sb = consts.tile([128, 2], BF16, tag="ones", name="ones_sb")
            nc.scalar.dma_start(out=ones_sb, in_=onesd)
            # Preload the exp table set while the PE is in phase 1.
            warm = consts.tile([128, 2], F32, tag="warm", name="warm")
            nc.scalar.activation(warm, ones_sb, AF.Exp)

            # xkT column tiles are dep-free -> keep their stream out of the
            # startup-critical am/xq DMA window.
            xkt = []
            for kt in range(K_T):
                w = xktpool.tile([128, 8, 128], BF16, tag="xkt", bufs=K_T,
                                 name=f"xkt{kt}")
                di = nc.sync.dma_start(
                    out=w, in_=xkP[kt].rearrange("p (a c) -> p a c", a=8))
                if kt == 0:
                    tile.add_dep_helper(
                        di.ins, xq_last_dma.ins,
                        reason="xkt stream after startup loads")
                xkt.append(w)

            # ---- H groups: for qc, m: accumulate over ec ----
            for qc in range(2):
                for m in range(8):
                    ps = pps.tile([128, 512], F32, tag="ps",
                                  name=f"psh{qc}_{m}")
                    for ec in range(8):
                        nc.tensor.matmul(
                            ps, am[m][:, ec, :], xq_slice(ec, qc),
                            start=(ec == 0), stop=(ec == 7))
                    nc.vector.tensor_scalar_add(
                        ht[m][:, qc * 512:(qc + 1) * 512], ps,
                        aCol_sb[:, m:m + 1])

        # ---------------- Phase 2: scores, softmax, values ----------------
        with tc.tile_pool(name="att", bufs=1) as at_p:
            # xkN row-tiles (resident through Z^T), then wv row-tiles;
            # both stream on sync behind xkt.
            xkn = []
            for kt in range(K_T):
                w = xknpool.tile([128, D], BF16, tag="xkn", bufs=K_T,
                                 name=f"xkn{kt}")
                di = nc.sync.dma_start(
                    out=w, in_=xkN[kt * 128:(kt + 1) * 128, :])
                if kt == 0:
                    tile.add_dep_helper(di.ins, xq_last_dma.ins,
                                        reason="xkn stream out of startup")
                xkn.append(w)
            wv = []
            for dp in range(8):
                w = wvpool.tile([128, D], BF16, tag="wv", bufs=8,
                                name=f"wv{dp}")
                nc.sync.dma_start(out=w, in_=wvT[dp * 128:(dp + 1) * 128, :])
                wv.append(w)

            # ---- S^T[k,q] = xkT.T @ H -> fused pad-bias+exp; sumexp ----
            at = [[None] * K_T for _ in range(2)]
            rbs = []
            for qc in range(2):
                for kt in range(K_T):
                    ps = pps.tile([128, 512], F32, tag="ps",
                                  name=f"pss{qc}_{kt}")
                    for dc in range(8):
                        nc.tensor.matmul(
                            ps, xkt[kt][:, dc, :],
                            ht[dc][:, qc * 512:(qc + 1) * 512],
                            start=(dc == 0), stop=(dc == 7))
                    a = at_p.tile([128, 512], BF16, tag="at", bufs=2 * K_T,
                                  name=f"at{qc}_{kt}")
                    nc.scalar.activation(
                        a, ps, AF.Exp,
                        bias=mb_sb[:, kt:kt + 1], scale=SCALE)
                    at[qc][kt] = a

                # sumexp -> reciprocal -> partition broadcast; overlaps the
                # next S^T pass / Z stream on PE.
                srow = pps.tile([2, 512], F32, tag="ps_sum", bufs=2,
                                name=f"srow{qc}")
                for kt in range(K_T):
                    nc.tensor.matmul(
                        srow, ones_sb, at[qc][kt],
                        start=(kt == 0), stop=(kt == K_T - 1))
                rrow = at_p.tile([2, 512], F32, tag="rrow", bufs=2,
                                 name=f"rrow{qc}")
                nc.vector.reciprocal(rrow[0:1, :], srow[0:1, :])
                rb = at_p.tile([128, 512], F32, tag="rb", bufs=2,
                               name=f"rb{qc}")
                nc.gpsimd.partition_broadcast(rb, rrow[0:1, :], channels=128)
                rbs.append(rb)

            # ---- per qc: Z^T then out^T (interleaved passes) ----
            for qc in range(2):
                zt = []
                for dt in range(8):
                    ps = pps.tile([128, 512], F32, tag="ps",
                                  name=f"psz{qc}_{dt}")
                    for kt in range(K_T):
                        nc.tensor.matmul(
                            ps, xkn[kt][:, dt * 128:(dt + 1) * 128],
                            at[qc][kt],
                            start=(kt == 0), stop=(kt == K_T - 1))
                    z = at_p.tile([128, 512], BF16, tag="zt", bufs=12,
                                  name=f"zt{qc}_{dt}")
                    nc.vector.tensor_mul(z, ps, rbs[qc])
                    zt.append(z)

                for dvt in range(8):
                    ps = pps.tile([128, 512], F32, tag="ps",
                                  name=f"pso{qc}_{dvt}")
                    for dp in range(8):
                        nc.tensor.matmul(
                            ps, wv[dp][:, dvt * 128:(dvt + 1) * 128],
                            zt[dp],
                            start=(dp == 0), stop=(dp == 7))
                    o = at_p.tile([128, 512], F32, tag="o", bufs=4,
                                  name=f"o{qc}_{dvt}")
                    weng = nc.scalar if qc == 0 else nc.sync
                    last = qc == 1 and dvt == 7
                    # Split the kernel's very last add+write chain so the
                    # final DMA covers half the bytes (shorter tail).
                    for piece in ([(0, 256), (256, 512)] if last
                                  else [(0, 512)]):
                        lo, hi = piece
                        nc.vector.tensor_scalar_add(
                            o[:, lo:hi], ps[:, lo:hi], bv_sb[:, dvt:dvt + 1])
                        weng.dma_start(
                            out=out_d[dvt * 128:(dvt + 1) * 128,
                                      qc * 512 + lo:qc * 512 + hi],
                            in_=o[:, lo:hi])


def _prep_inputs(x, mask, Wq, bq, Wk, bk, Wv, bv):
    x = np.ascontiguousarray(np.asarray(x, dtype=np.float32))
    mask = np.asarray(mask, dtype=bool)
    Wq = np.asarray(Wq, dtype=np.float64)
    bq = np.asarray(bq, dtype=np.float64)
    Wk = np.asarray(Wk, dtype=np.float64)
    Wv = np.asarray(Wv, dtype=np.float32)
    bv = np.asarray(bv, dtype=np.float32)
    del bk  # exactly cancelled by softmax shift invariance

    A = (Wq.T @ Wk).astype(np.float32)
    a_vec = (bq @ Wk).astype(np.float32)
    # aP[m, p, a*128+c] = A[a*128+p, m*128+c]
    aP = np.ascontiguousarray(
        A.reshape(8, 128, 8, 128).transpose(2, 1, 0, 3)
        .reshape(8, 128, D).astype(NP_BF16))
    wvT = np.ascontiguousarray(Wv.T.astype(NP_BF16))
    aColT = np.ascontiguousarray(a_vec.reshape(8, 128).T)
    bvT = np.ascontiguousarray(bv.reshape(8, 128).T)
    ones = np.ones((128, 2), dtype=NP_BF16)

    counts = mask.sum(axis=1)
    K_T = int(np.ceil(counts.max() / 128))
    K_pad = K_T * 128

    in_maps = []
    for c in range(N_CORES):
        b, h = divmod(c, 2)
        sel = np.where(mask[b])[0]
        K = len(sel)
        xk = np.zeros((K_pad, D), dtype=np.float32)
        xk[:K] = x[b, sel]
        mb = np.zeros(K_pad, dtype=np.float32)
        mb[K:] = MASK_NEG
        mbT = np.ascontiguousarray(mb.reshape(K_T, 128).T)
        xq_c = np.ascontiguousarray(
            x[b, h * QL:(h + 1) * QL].T.astype(NP_BF16))
        # xkP[kt, p, dc*128+c] = xk[kt*128+c, dc*128+p]
        xkP = np.ascontiguousarray(
            xk.reshape(K_T, 128, 8, 128).transpose(0, 3, 2, 1)
            .reshape(K_T, 128, D).astype(NP_BF16))
        in_maps.append({
            "aP": aP, "xqT": xq_c, "xkP": xkP,
            "xkN": np.ascontiguousarray(xk.astype(NP_BF16)),
            "wvT": wvT,
            "aCol": aColT, "mbT": mbT, "bvT": bvT, "onesd": ones,
        })
    return in_maps, K_T


def run(x, mask, Wq, bq, Wk, bk, Wv, bv, trace=False):
    """Build + run; returns (output, BassKernelResults)."""
    in_maps, K_T = _prep_inputs(x, mask, Wq, bq, Wk, bk, Wv, bv)
    nc = _build_nc(K_T)
    res = run_bass_kernel_spmd(nc, in_maps, list(range(N_CORES)), trace=trace)
    out = np.empty((4, 2048, D), dtype=np.float32)
    for c in range(N_CORES):
        b, h = divmod(c, 2)
        out[b, h * QL:(h + 1) * QL, :] = res.results[c]["out"].T
    return out, res


def kernel(x, mask, Wq, bq, Wk, bk, Wv, bv):
    out, _ = run(x, mask, Wq, bq, Wk, bk, Wv, bv)
    return out
